# revision 17
# baseline (speedup 1.0000x reference)
"""DenseAtt kernel for Trainium2 (Bass/Tile), 8-core batch-parallel.

Math (per batch element b):
    s_left  = x @ W[:d]          # [n]
    s_right = x @ W[d:]          # [n]
    out[i,j] = sigmoid(s_left[i] + s_right[j] + bias) * adj[i,j]

Shapes: x [8, 2048, 128] f32, adj [8, 2048, 2048] f32, W [256] f32, b [] f32.
Sharding: one batch element per NeuronCore (B == n_cores == 8), no
collectives; full inputs in, full output out, gather on host.

The kernel is DMA-bandwidth-bound: per core it must read adj (one n x n
panel) and write out (same shape), sharing one ~360 GB/s DMA bus. To halve
the bus bytes, adj/x are converted to bf16 on the host and out is produced
in bf16 on device, upcast to f32 on the host. Each rounding step is ~2^-9
relative, far inside the 2e-2 tolerance; the sigmoid argument s_left/
s_right stays f32 on device (PE accumulates in f32).

Device plan per core:
  1. xT (x transposed on host -> [d, n] bf16) lands in SBUF via one
     contiguous DMA; it is the layout every PE matmul wants.
  2. PE matmuls: s_right row chunks = wc[:,1].T @ xT_i -> [1, 512] PSUM,
     ACT copies them into sr_row; s_left block i = xT_i.T @ wc[:,0] ->
     [128, 16] PSUM, ACT folds the scalar bias while copying out.
  3. s_right row [1, 2048] -> gpsimd.partition_broadcast -> sr_b [128, 2048].
  4. Main loop over row blocks:
       adj_t <- DMA bf16 pair-block       (SP HWDGE queue, deep prefetch)
       att_t <- ACT sigmoid(sr_b + bias=s_left[:, i]) -> bf16  (per block)
       out_t <- DVE att_t * adj_t         (bf16, 2x DVE mode)
       DMA out bf16 block                 (Pool SWDGE queue, per block)
     adj ins sit alone on the SP queue so the whole input stream prefetches
     back-to-back from t=0 (adj_bufs holds the full panel); out DMAs go
     through the Pool queue so their compute-waits never head-block the
     input stream. att/out pools are deep enough that the sigmoid stream
     never stalls while the bus is still busy with the input stream.
"""

from contextlib import ExitStack

import numpy as np

import concourse.bass as bass
import concourse.tile as tile
from concourse import bacc, mybir
from concourse.bass_utils import run_bass_kernel_spmd

N = 2048
D = 128
P = 128
NBLK = N // P  # 16
NCORES = 8

_cache = {}


def _build(
    adj_bufs=8,
    att_bufs=4,
    out_bufs=12,
    blocks_per_iter=2,
    out_bpi=1,  # granularity of mult + out DMA (<= blocks_per_iter)
    warm_act=True,
    out_eng="gp",  # engine issuing the out DMAs: "gp" | "act" | "sp"
    repeat_full=1,  # timing: chain the ENTIRE kernel (setup included) M times
) -> bass.Bass:
    f32 = mybir.dt.float32
    bf16 = mybir.dt.bfloat16
    nc = bacc.Bacc("TRN2", target_bir_lowering=False, debug=False)

    xt = nc.dram_tensor("xt", [D, N], bf16, kind="ExternalInput").ap()
    adj = nc.dram_tensor("adj", [N, N], bf16, kind="ExternalInput").ap()
    bb = nc.dram_tensor("bb", [P, 1], f32, kind="ExternalInput").ap()
    wc = nc.dram_tensor("wc", [P, 2], bf16, kind="ExternalInput").ap()
    out = nc.dram_tensor("out", [N, N], bf16, kind="ExternalOutput").ap()

    with ExitStack() as ctx:
        tc = ctx.enter_context(tile.TileContext(nc))
        const = ctx.enter_context(tc.tile_pool(name="const", bufs=1))
        rot = ctx.enter_context(
            tc.tile_pool(name="rot", bufs=2 if repeat_full > 1 else 1)
        )
        adj_pool = ctx.enter_context(tc.tile_pool(name="adjp", bufs=adj_bufs))
        att_pool = ctx.enter_context(tc.tile_pool(name="attp", bufs=att_bufs))
        out_pool = ctx.enter_context(tc.tile_pool(name="outp", bufs=out_bufs))

        bb_t = const.tile([P, 1], f32)
        wc_t = const.tile([P, 2], bf16)
        slp_pool = ctx.enter_context(tc.tile_pool(name="slp", bufs=2, space="PSUM"))
        srp_pool = ctx.enter_context(tc.tile_pool(name="srp", bufs=4, space="PSUM"))

        def out_dma_for(blk):
            if out_eng == "gp_sp":  # alternate queues per block
                return nc.gpsimd.dma_start if blk % 2 == 0 else nc.sync.dma_start
            return {
                "gp": nc.gpsimd.dma_start,
                "act": nc.scalar.dma_start,
                "sp": nc.sync.dma_start,
            }[out_eng]

        for _rep in range(repeat_full):
            # xT goes on the bus FIRST (it gates the whole setup chain).
            # The first adj block is issued immediately after it, BEFORE the
            # tiny wc/bb loads: the HWDGE desc-gen pipeline (625ns/DMA) would
            # otherwise gate the first adj transfer and idle the bus.
            xt_t = rot.tile([P, N], bf16, tag="xt")  # xT: [d, (i n)]
            nc.sync.dma_start(xt_t[:], xt)

            if _rep == 0:
                if warm_act:
                    # Load the sigmoid ACT table at t=0, off the critical
                    # path.
                    warm = const.tile([P, 1], f32)
                    nc.vector.memset(warm[:], 0.0)
                    nc.scalar.activation(
                        warm[:], warm[:], mybir.ActivationFunctionType.Sigmoid
                    )

            # --- issue the whole adj in-stream up front on the SP queue ---
            BPI = blocks_per_iter
            W_ = N * BPI
            adj_tiles = []
            for it in range(NBLK // BPI):
                i0 = it * BPI
                adj_t = adj_pool.tile([P, W_], bf16)
                if BPI == 1:
                    nc.sync.dma_start(adj_t[:], adj[i0 * P : (i0 + 1) * P, :])
                else:
                    nc.sync.dma_start(
                        adj_t[:].rearrange("p (u j) -> p u j", u=BPI),
                        adj[i0 * P : (i0 + BPI) * P, :].rearrange(
                            "(u p) j -> p u j", p=P
                        ),
                    )
                adj_tiles.append(adj_t)
                if it == 0 and _rep == 0:
                    nc.sync.dma_start(wc_t[:], wc)
                    nc.sync.dma_start(bb_t[:], bb)

            sl_t = rot.tile([P, NBLK], f32, tag="sl")  # s_left + b
            sr_b = rot.tile([P, N], f32, tag="srb")  # s_right bcast

            # s_right row chunks: [1, 128] per block -> 4x [1, 512]
            sr_row = rot.tile([1, N], f32, tag="sr_row")
            for c in range(4):
                src = srp_pool.tile([1, 4 * P], f32)
                for o in range(4):
                    i = c * 4 + o
                    nc.tensor.matmul(
                        src[:, o * P : (o + 1) * P],
                        wc_t[:, 1:2],
                        xt_t[:, i * P : (i + 1) * P],
                    )
                nc.scalar.copy(sr_row[:, c * 4 * P : (c + 1) * 4 * P], src[:])
            nc.gpsimd.partition_broadcast(sr_b[:], sr_row[:])

            # s_left columns [128, 16], bias b folded in the ACT copy
            sl_ps = slp_pool.tile([P, NBLK], f32)
            for i in range(NBLK):
                nc.tensor.matmul(
                    sl_ps[:, i : i + 1],
                    xt_t[:, i * P : (i + 1) * P],
                    wc_t[:, 0:1],
                )
            nc.scalar.add(sl_t[:], sl_ps[:], bb_t[:, 0:1])

            # --- main loop over row blocks (bf16 streams) ---
            # adj arrived in blocks_per_iter row-blocks per DMA; the
            # sigmoid/mult/out side runs at out_bpi granularity so the
            # tail after the last sigmoid is short.
            OB = out_bpi
            assert BPI % OB == 0
            OW = N * OB
            for it in range(NBLK // BPI):
                i0 = it * BPI
                adj_t = adj_tiles[it]
                for v in range(BPI // OB):
                    j0 = i0 + v * OB
                    att_t = att_pool.tile([P, OW], bf16)
                    for u in range(OB):
                        nc.scalar.activation(
                            att_t[:, u * N : (u + 1) * N],
                            sr_b[:],
                            mybir.ActivationFunctionType.Sigmoid,
                            bias=sl_t[:, j0 + u : j0 + u + 1],
                            scale=1.0,
                        )
                    o_t = out_pool.tile([P, OW], bf16)
                    nc.vector.tensor_tensor(
                        o_t[:],
                        att_t[:],
                        adj_t[:, v * OW : (v + 1) * OW],
                        op=mybir.AluOpType.mult,
                    )
                    out_dma = out_dma_for(j0)
                    if OB == 1:
                        out_dma(out[j0 * P : (j0 + 1) * P, :], o_t[:])
                    else:
                        out_dma(
                            out[j0 * P : (j0 + OB) * P, :].rearrange(
                                "(u p) j -> p u j", p=P
                            ),
                            o_t[:].rearrange("p (u j) -> p u j", u=OB),
                        )

    nc.compile()
    return nc


PROD_CONFIG = dict(
    adj_bufs=8,
    att_bufs=4,
    out_bufs=12,
    blocks_per_iter=2,
    out_bpi=1,
    warm_act=True,
    out_eng="gp_sp",
)


def _get_nc() -> bass.Bass:
    if "nc" not in _cache:
        _cache["nc"] = _build(**PROD_CONFIG)
    return _cache["nc"]


def _bf16(a):
    import ml_dtypes

    return np.ascontiguousarray(a.astype(ml_dtypes.bfloat16))


def _in_maps(x, adj, W, b):
    x = np.asarray(x, dtype=np.float32)
    adj = np.asarray(adj, dtype=np.float32)
    W = np.asarray(W, dtype=np.float32)
    b = np.float32(np.asarray(b, dtype=np.float32))
    wc = _bf16(W.reshape(2, D).T)
    bbv = np.full((P, 1), b, dtype=np.float32)
    shared = {"wc": wc, "bb": bbv}
    return [
        {
            "xt": _bf16(x[c].T),
            "adj": _bf16(adj[c]),
            **shared,
        }
        for c in range(NCORES)
    ]


def run(x, adj, W, b, trace=False):
    import os

    if not trace:
        # This axon client image has no NTFF profile hook
        # (antenv.axon_hooks); an inherited BASS_TRACE=1 would crash the
        # run on that import, so force tracing off.
        os.environ["BASS_NEVER_TRACE"] = "1"
    nc = _get_nc()
    res = run_bass_kernel_spmd(
        nc,
        _in_maps(x, adj, W, b),
        core_ids=list(range(NCORES)),
        trace=trace,
    )
    out = np.stack(
        [res.results[c]["out"].astype(np.float32) for c in range(NCORES)], axis=0
    )
    return out, res


def kernel(x, adj, W, b):
    out, _ = run(x, adj, W, b)
    return out
